# revision 1
# baseline (speedup 1.0000x reference)
"""CLIP-style contrastive train loss on Trainium2 (Bass/Tile, 8 NeuronCores).

Problem (hardcoded shapes):
  skeleton_embeddings: [32, 120, 64, 512] f32
  text_embeddings:     [32, 120, 512]     f32
  out: scalar f32 loss = -mean_{b,m} log_softmax(S * text_f @ skel_f^T)[m, m]
  where skel = mean_t(skeleton), both L2-normalized over d, S = 1/0.07.

Sharding: data-parallel over the batch dim (4 batches per core, 8 cores).
Each core emits per-row loss terms v[m, b] = lse[m] - logits[m, m]; the host
sums all 8 cores' [120, 4] partials and divides by 32*120.

Key structure (memory-bound problem; ~63 MB/core of skeleton dominates):
 - skeleton streams in [120, 8, 512] f32 slabs (HWDGE, contiguous per row);
   pooling over t runs on the vector engine as chained strided reduces —
   each slab tile carries one extra t-slot holding the running partial, so
   no separate adds are needed and DVE stays just under the DMA rate.
 - The 1/64 mean divisor cancels inside L2 normalization (plain sum pool).
 - LOGIT_SCALE folds into the text normalization factor; the skeleton-side
   normalization is factored out of the matmul entirely: G_raw uses the raw
   pooled skeleton, and logits = G_raw * SCL where SCL[m,n] = rs_s[n] is a
   rank-1 matrix built by a K=1 matmul (ones_row^T @ rs_row).  This takes
   the norm chain off the transpose/matmul critical path at the kernel tail.
 - 1/sqrt(x) is computed as exp(-0.5*ln(x)): all ACT functions used
   (Square/Ln/Exp/Copy) then live in ONE activation-table set, so the
   scalar engine loads its table exactly once (see _patch_act_tables).
 - The last slab of the last batch is split into d-quarter DMAs + reduces
   feeding the per-chunk transposes, shortening the post-last-byte tail.
"""

import functools
from contextlib import ExitStack

import numpy as np

import concourse.bass as bass
import concourse.tile as tile
from concourse import bacc, mybir
from concourse.bass_utils import run_bass_kernel_spmd


class _patched_act_tables:
    """Context manager restricting the ACT-table chooser to the one set that
    contains every function this kernel uses (square/ln/exp/copy/identity),
    so the scalar engine loads its table once instead of ping-ponging
    between the exp-only and ln-only sets on every batch.  Restores the
    original chooser on exit so no global state leaks."""

    def __enter__(self):
        import concourse.hw_specs as hw_specs

        self._hw_specs = hw_specs
        self._real = hw_specs.get_activation_tables
        self._bacc_real = bacc.get_activation_tables
        real = self._real

        @functools.cache
        def only_full_set(arch):
            tabs = real(arch)
            return {
                name: (funcs if name == "natural_log_exp_and_others" else set())
                for name, funcs in tabs.items()
            }

        hw_specs.get_activation_tables = only_full_set
        bacc.get_activation_tables = only_full_set
        return self

    def __exit__(self, *exc):
        self._hw_specs.get_activation_tables = self._real
        bacc.get_activation_tables = self._bacc_real
        return False


B, M, T, D = 32, 120, 64, 512
NCORES = 8
BPC = B // NCORES  # batches per core
TQ = 8             # t-chunk per DMA slab
LOGIT_SCALE = float(np.exp(np.log(1.0 / 0.07)))

FP32 = mybir.dt.float32
F32R = mybir.dt.float32r
AF = mybir.ActivationFunctionType
OP = mybir.AluOpType
AX = mybir.AxisListType

# float32r = single-pass fp32 on the PE (vs 2-pass float32): 2x fewer cycles
# per row for the logits matmul.  Measured on HW: loss rel err 9.5e-7 (vs
# 6.6e-7 full fp32), per-row 3.9e-4 — effectively free for this loss.
USE_F32R = True


def _mm(ap):
    return ap.bitcast(F32R) if USE_F32R else ap


def _emit(tc, ctx, skel, text, ident, out):
    nc = tc.nc
    slabs = ctx.enter_context(tc.tile_pool(name="slabs", bufs=6))
    work = ctx.enter_context(tc.tile_pool(name="work", bufs=2))
    small = ctx.enter_context(tc.tile_pool(name="small", bufs=3))
    singles = ctx.enter_context(tc.tile_pool(name="singles", bufs=1))
    sbt = ctx.enter_context(tc.tile_pool(name="sbt", bufs=5))
    psum_t = ctx.enter_context(tc.tile_pool(name="psum_t", bufs=4, space="PSUM"))
    psum_g = ctx.enter_context(tc.tile_pool(name="psum_g", bufs=2, space="PSUM"))
    psum_x = ctx.enter_context(tc.tile_pool(name="psum_x", bufs=1, space="PSUM"))

    ident_sb = singles.tile([M, M], FP32, tag="ident")
    nc.sync.dma_start(ident_sb[:], ident[:, :])
    # Per-row loss terms for all local batches; one DMA out at the end.
    vacc = singles.tile([M, BPC], FP32, tag="vacc")

    LN_S = float(np.log(LOGIT_SCALE))
    lns_bias = singles.tile([M, 1], FP32, tag="lns_bias")
    nc.vector.memset(lns_bias[:], LN_S)
    ones_f = singles.tile([1, M], FP32, tag="ones_f")
    nc.vector.memset(ones_f[:], 1.0)
    # f32r consumers need an explicitly-rounded producer; a DVE copy is one.
    ones_row = singles.tile([1, M], FP32, tag="ones_row")
    nc.vector.tensor_copy(_mm(ones_row[:]), ones_f[:])
    nch = D // 128

    for b in range(BPC):
        # ---- text side first: no dependence on the skeleton stream --------
        txt = work.tile([M, D], FP32, tag="txt")
        nc.sync.dma_start(txt[:], text[b, :, :])
        sq_t = work.tile([M, D], FP32, tag="sq_t")
        st_t = small.tile([M, 1], FP32, tag="st_t")
        nc.scalar.activation(sq_t[:], txt[:], AF.Square, accum_out=st_t[:])
        ln_t = small.tile([M, 1], FP32, tag="ln_t")
        nc.scalar.activation(ln_t[:], st_t[:], AF.Ln)
        # rs_t = S / sqrt(st) = exp(-0.5*ln(st) + ln(S)): LOGIT_SCALE folded
        # into the text normalization so logits come out of the matmul scaled.
        rs_t = small.tile([M, 1], FP32, tag="rs_t")
        nc.scalar.activation(rs_t[:], ln_t[:], AF.Exp, scale=-0.5,
                             bias=lns_bias[:])
        txf = work.tile([M, D], FP32, tag="txf")
        nc.vector.tensor_scalar_mul(txf[:], txt[:], rs_t[:])
        t_chunks = []
        for c in range(nch):
            pt = psum_t.tile([128, M], FP32, tag="pt")
            nc.tensor.transpose(pt[:], txf[:, c * 128:(c + 1) * 128],
                                ident_sb[:])
            tT = sbt.tile([128, M], FP32, tag="tT")
            nc.scalar.copy(_mm(tT[:]), pt[:])
            t_chunks.append(tT)

        # ---- skeleton pooling over t (chained strided reduces) ------------
        nchunk = T // TQ
        last = b == BPC - 1
        slabs_b = []
        t0 = 0
        for h in range(nchunk):
            ts = 1 if h > 0 else 0  # slot 0 reserved for the running partial
            slab = slabs.tile([M, TQ + 1, D], FP32, tag="slab")
            if last and h == nchunk - 1:
                # d-quarter DMAs: each quarter's reduce + transpose can start
                # as soon as that quarter lands (shortens the exposed tail).
                for q in range(nch):
                    dq = slice(q * 128, (q + 1) * 128)
                    nc.sync.dma_start(slab[:, ts:ts + TQ, dq],
                                      skel[b, :, t0:t0 + TQ, dq])
            else:
                nc.sync.dma_start(slab[:, ts:ts + TQ, :],
                                  skel[b, :, t0:t0 + TQ, :])
            slabs_b.append(slab)
            t0 += TQ

        ssum = work.tile([M, D], FP32, tag="ssum")
        st_s = small.tile([M, 1], FP32, tag="st_s")
        sq_s = work.tile([M, D], FP32, tag="sq_s")
        G = psum_g.tile([M, M], FP32, tag="G")

        def skel_chunk(c):
            """Transpose raw pooled-skeleton chunk c and fold it into G."""
            ps = psum_t.tile([128, M], FP32, tag="pt", name="ps")
            sl = slice(c * 128, (c + 1) * 128)
            nc.tensor.transpose(ps[:], ssum[:, sl], ident_sb[:])
            sT = sbt.tile([128, M], FP32, tag="sT", name="sT")
            nc.scalar.copy(_mm(sT[:]), ps[:])
            nc.tensor.matmul(
                G[:], _mm(t_chunks[c][:]), _mm(sT[:]),
                start=(c == 0), stop=(c == nch - 1),
            )

        for h, slab in enumerate(slabs_b):
            # h=0 has no partial slot: data lives in slots [0, TQ); later
            # chunks read slots [0, TQ+1) = running partial + new data.
            hi = TQ if h == 0 else TQ + 1
            if last and h == nchunk - 1:
                # per-d-quarter reduce -> square -> transpose -> G, pipelined;
                # quarter norms land in columns of one tile and are merged by
                # a single ACT copy-with-accumulate (no DVE adds on the tail).
                sth4 = small.tile([M, nch], FP32, tag="sth4")
                s4_scr = small.tile([M, nch], FP32, tag="s4_scr")
                for q in range(nch):
                    dq = slice(q * 128, (q + 1) * 128)
                    src = slab[:, 0:hi, dq].rearrange("n t d -> n d t")
                    nc.vector.reduce_sum(ssum[:, dq], src, axis=AX.X)
                    nc.scalar.activation(sq_s[:, dq], ssum[:, dq], AF.Square,
                                         accum_out=sth4[:, q:q + 1])
                    skel_chunk(q)
                nc.scalar.activation(s4_scr[:], sth4[:], AF.Copy,
                                     accum_out=st_s[:])
            else:
                dst = slabs_b[h + 1][:, 0, :] if h + 1 < nchunk else ssum[:]
                src = slab[:, 0:hi, :].rearrange("n t d -> n d t")
                nc.vector.reduce_sum(dst, src, axis=AX.X)
        if not last:
            nc.scalar.activation(sq_s[:], ssum[:], AF.Square,
                                 accum_out=st_s[:])
            for c in range(nch):
                skel_chunk(c)

        # ---- rs_s = 1/sqrt(st) = exp(-0.5*ln(st)) --------------------------
        ln_s = small.tile([M, 1], FP32, tag="ln_s")
        nc.scalar.activation(ln_s[:], st_s[:], AF.Ln)
        rs_s = small.tile([M, 1], FP32, tag="rs_s")
        nc.scalar.activation(rs_s[:], ln_s[:], AF.Exp, scale=-0.5)
        # SCL[m, n] = rs_s[n]: rank-1 broadcast via transpose + K=1 matmul.
        pr = psum_x.tile([1, M], FP32, tag="pr")
        nc.tensor.transpose(pr[:], rs_s[:], ident_sb[:])
        rs_row = small.tile([1, M], FP32, tag="rs_row")
        nc.scalar.copy(_mm(rs_row[:]), pr[:])
        scl_ps = psum_x.tile([M, M], FP32, tag="scl_ps")
        nc.tensor.matmul(scl_ps[:], _mm(ones_row[:]), _mm(rs_row[:]),
                         start=True, stop=True)
        scl = work.tile([M, M], FP32, tag="scl")
        nc.vector.tensor_copy(scl[:], scl_ps[:])

        # ---- logits u = G_raw * SCL; row logsumexp (|u| <= ~14.3) ----------
        u = work.tile([M, M], FP32, tag="u")
        nc.vector.tensor_tensor(u[:], G[:], scl[:], op=OP.mult)
        e_scr = work.tile([M, M], FP32, tag="e_scr")
        se = small.tile([M, 1], FP32, tag="se")
        nc.scalar.activation(e_scr[:], u[:], AF.Exp, accum_out=se[:])
        lse = small.tile([M, 1], FP32, tag="lse")
        nc.scalar.activation(lse[:], se[:], AF.Ln)

        # ---- diag(u) via identity mask; v = lse - diag ---------------------
        gd_scr = work.tile([M, M], FP32, tag="gd_scr")
        gd = small.tile([M, 1], FP32, tag="gd")
        nc.vector.scalar_tensor_tensor(
            gd_scr[:], u[:], 1.0, ident_sb[:],
            op0=OP.mult, op1=OP.mult, accum_out=gd[:],
        )
        nc.vector.tensor_tensor(
            vacc[:, b:b + 1], lse[:], gd[:], op=OP.subtract
        )

    nc.sync.dma_start(out[:, :], vacc[:])


def _build_nc():
    nc = bacc.Bacc("TRN2", debug=False)
    skel = nc.dram_tensor("skel", [BPC, M, T, D], FP32, kind="ExternalInput")
    text = nc.dram_tensor("text", [BPC, M, D], FP32, kind="ExternalInput")
    ident = nc.dram_tensor("ident", [M, M], FP32, kind="ExternalInput")
    out = nc.dram_tensor("partial", [M, BPC], FP32, kind="ExternalOutput")
    with tile.TileContext(nc) as tc, ExitStack() as ctx:
        _emit(tc, ctx, skel.ap(), text.ap(), ident.ap(), out.ap())
    with _patched_act_tables():
        nc.compile()
    return nc


_NC_CACHE = []


def _run(skeleton_embeddings, text_embeddings, **kw):
    if not _NC_CACHE:
        _NC_CACHE.append(_build_nc())
    nc = _NC_CACHE[0]
    skel = np.ascontiguousarray(np.asarray(skeleton_embeddings, dtype=np.float32))
    text = np.ascontiguousarray(np.asarray(text_embeddings, dtype=np.float32))
    ident = np.eye(M, dtype=np.float32)
    in_maps = [
        {
            "skel": skel[c * BPC:(c + 1) * BPC],
            "text": text[c * BPC:(c + 1) * BPC],
            "ident": ident,
        }
        for c in range(NCORES)
    ]
    r = run_bass_kernel_spmd(nc, in_maps, core_ids=list(range(NCORES)), **kw)
    total = sum(float(m["partial"].sum()) for m in r.results)
    loss = np.float32(total / (B * M))
    return loss, r


def kernel(skeleton_embeddings, text_embeddings):
    loss, _ = _run(skeleton_embeddings, text_embeddings)
    return np.asarray(loss, dtype=np.float32)



# revision 6
# speedup vs baseline: 1.0020x; 1.0020x over previous
"""CLIP-style contrastive train loss on Trainium2 (Bass/Tile, 8 NeuronCores).

Problem (hardcoded shapes):
  skeleton_embeddings: [32, 120, 64, 512] f32
  text_embeddings:     [32, 120, 512]     f32
  out: scalar f32 loss = -mean_{b,m} log_softmax(S * text_f @ skel_f^T)[m, m]
  where skel = mean_t(skeleton), both L2-normalized over d, S = 1/0.07.

Sharding: data-parallel over the batch dim (4 batches per core, 8 cores).

Structure (memory-bound; ~63 MB/core of skeleton => DMA bus is the floor at
360 B/ns; everything else hides under the stream except the head and tail):
 - skeleton streams in [120, k, 512] f32 slabs; pooling over t runs on DVE as
   chained strided reduces - each slab carries one extra t-slot holding the
   running partial, so no separate adds are needed.
 - The 1/64 mean divisor cancels inside L2 normalization (plain sum pool).
 - LOGIT_SCALE folds into the text normalization factor.
 - Logits are built TRANSPOSED: GT[n,m] = sum_d sT[d,n] * tT[d,m] accumulated
   in PSUM, so the skeleton-side norm scale rs_s[n] is a per-PARTITION scale:
   E = exp(rs_s * GT) is a single fused ACT op reading PSUM directly.
 - Row sums sum_n E[n,m] become a PE ones-matmul -> se_row [1, M]; the host
   does ln(se), the diag term rs[m]*GT[m,m], and all the final sums, so the
   device tail ends at the ones-matmul + one tiny DMA.
 - Last batch uses a DECREASING slab schedule [10,10,10,9,8,7,5,3] + a final
   2-t-slice slab split into 4 d-quarter DMAs merged by 3-slot reduces, so
   DVE has ~0.5us (not ~5us) of queued reduce work when the last byte lands.
 - 1/sqrt(x) is computed as exp(-0.5*ln(x)): all ACT functions used
   (Square/Ln/Exp/Copy) then live in ONE activation-table set, so the
   scalar engine loads its table exactly once (see _patch_act_tables).
"""

import functools
from contextlib import ExitStack

import numpy as np

import concourse.bass as bass
import concourse.tile as tile
from concourse import bacc, mybir
from concourse.bass_utils import run_bass_kernel_spmd


class _patched_act_tables:
    """Context manager restricting the ACT-table chooser to the one set that
    contains every function this kernel uses (square/ln/exp/copy/identity),
    so the scalar engine loads its table once instead of ping-ponging
    between the exp-only and ln-only sets on every batch.  Restores the
    original chooser on exit so no global state leaks."""

    def __enter__(self):
        import concourse.hw_specs as hw_specs

        self._hw_specs = hw_specs
        self._real = hw_specs.get_activation_tables
        self._bacc_real = bacc.get_activation_tables
        real = self._real

        @functools.cache
        def only_full_set(arch):
            tabs = real(arch)
            return {
                name: (funcs if name == "natural_log_exp_and_others" else set())
                for name, funcs in tabs.items()
            }

        hw_specs.get_activation_tables = only_full_set
        bacc.get_activation_tables = only_full_set
        return self

    def __exit__(self, *exc):
        self._hw_specs.get_activation_tables = self._real
        bacc.get_activation_tables = self._bacc_real
        return False


B, M, T, D = 32, 120, 64, 512
NCORES = 8
BPC = B // NCORES  # batches per core
LOGIT_SCALE = float(np.exp(np.log(1.0 / 0.07)))

FP32 = mybir.dt.float32
F32R = mybir.dt.float32r
AF = mybir.ActivationFunctionType
OP = mybir.AluOpType
AX = mybir.AxisListType

# Uniform slab schedule for batches 0..BPC-2 (tail hidden under next batch's
# stream) and a decreasing schedule for the last batch (minimizes DVE work
# still queued when the final byte lands).  Both sum to T - LAST_T.
LAST_T = 2          # final t-slices, DMA'd as 4 d-quarter pieces
SCHED_MID = [8] * 7 + [6]
SCHED_LAST = [10, 10, 10, 9, 8, 7, 5, 3]
assert sum(SCHED_MID) == sum(SCHED_LAST) == T - LAST_T

# float32r = single-pass fp32 on the PE (vs 2-pass float32): 2x fewer cycles
# per row for the logits matmul.  Loss rel err ~1e-6 - free for this loss.
USE_F32R = True


def _mm(ap):
    return ap.bitcast(F32R) if USE_F32R else ap


def _emit(tc, ctx, skel, text, ident, seR, rg):
    nc = tc.nc
    slabs = ctx.enter_context(tc.tile_pool(name="slabs", bufs=6))
    qpool = ctx.enter_context(tc.tile_pool(name="qpool", bufs=2))
    work = ctx.enter_context(tc.tile_pool(name="work", bufs=2))
    small = ctx.enter_context(tc.tile_pool(name="small", bufs=3))
    singles = ctx.enter_context(tc.tile_pool(name="singles", bufs=1))
    sbt = ctx.enter_context(tc.tile_pool(name="sbt", bufs=8))
    psum_t = ctx.enter_context(tc.tile_pool(name="psum_t", bufs=4, space="PSUM"))
    psum_g = ctx.enter_context(tc.tile_pool(name="psum_g", bufs=2, space="PSUM"))
    psum_se = ctx.enter_context(tc.tile_pool(name="psum_se", bufs=2, space="PSUM"))
    KMAX = max(max(SCHED_MID), max(SCHED_LAST))

    ident_sb = singles.tile([M, 128], FP32, tag="ident")
    # rs_s / diag(GT) per batch, summed on host: col 2b = rs, col 2b+1 = diag.
    vrg = singles.tile([M, 2 * BPC], FP32, tag="vrg")

    LN_S = float(np.log(LOGIT_SCALE))
    lns_bias = singles.tile([M, 1], FP32, tag="lns_bias")
    ones_f = singles.tile([M, 1], FP32, tag="ones_f")
    ones_col = singles.tile([M, 1], FP32, tag="ones_col")
    nch = D // 128

    for b in range(BPC):
        last = b == BPC - 1
        sched = SCHED_LAST if last else SCHED_MID

        # ---- DMA order: batch 0 leads with a fat slab (head latency hides
        # the small singles/text transfers under it); later batches lead with
        # text so it's in-flight earliest for the tT chain.
        txt = work.tile([M, D], FP32, tag="txt")
        slabs_b = []
        t0 = 0

        def slab_dma(h, k, t0):
            ts = 1 if h > 0 else 0  # slot 0 reserved for the running partial
            slab = slabs.tile([M, KMAX + 1, D], FP32, tag="slab")
            nc.sync.dma_start(slab[:, ts:ts + k, :], skel[b, :, t0:t0 + k, :])
            slabs_b.append((slab, k))

        if b == 0:
            slab_dma(0, sched[0], 0)
            t0 = sched[0]
            nc.sync.dma_start(ident_sb[:], ident[:, :])
            nc.vector.memset(lns_bias[:], LN_S)
            nc.vector.memset(ones_f[:], 1.0)
            # f32r consumers need an explicitly-rounded producer; a DVE copy
            # is one.
            nc.vector.tensor_copy(_mm(ones_col[:]), ones_f[:])
        nc.sync.dma_start(txt[:], text[b, :, :])
        for h in range(1 if b == 0 else 0, len(sched)):
            slab_dma(h, sched[h], t0)
            t0 += sched[h]
        # final LAST_T t-slices: slot 0 = partial, slots 1:1+LAST_T = data,
        # DMA'd per d-quarter so the 4 closing reduces/transposes pipeline.
        qslab = qpool.tile([M, 1 + LAST_T, D], FP32, tag="qslab")
        for c in range(nch):
            dq = slice(c * 128, (c + 1) * 128)
            nc.sync.dma_start(qslab[:, 1:1 + LAST_T, dq],
                              skel[b, :, t0:t0 + LAST_T, dq])

        # ---- text side: no dependence on the skeleton stream --------------
        sq_t = work.tile([M, D], FP32, tag="sq_t")
        st_t = small.tile([M, 1], FP32, tag="st_t")
        nc.scalar.activation(sq_t[:], txt[:], AF.Square, accum_out=st_t[:])
        ln_t = small.tile([M, 1], FP32, tag="ln_t")
        nc.scalar.activation(ln_t[:], st_t[:], AF.Ln)
        # rs_t = S / sqrt(st) = exp(-0.5*ln(st) + ln(S)): LOGIT_SCALE folded
        # into the text normalization so logits come out of the matmul scaled.
        rs_t = small.tile([M, 1], FP32, tag="rs_t")
        nc.scalar.activation(rs_t[:], ln_t[:], AF.Exp, scale=-0.5,
                             bias=lns_bias[:])
        txf = work.tile([M, D], FP32, tag="txf")
        nc.vector.tensor_scalar_mul(txf[:], txt[:], rs_t[:])
        t_chunks = []
        for c in range(nch):
            pt = psum_t.tile([128, M], FP32, tag="pt")
            nc.tensor.transpose(pt[:], txf[:, c * 128:(c + 1) * 128],
                                ident_sb[:, 0:M])
            tT = sbt.tile([128, M], FP32, tag="tT")
            nc.scalar.copy(_mm(tT[:]), pt[:])
            t_chunks.append(tT)

        # ---- pooling over t: chained strided reduces on DVE ---------------
        for h, (slab, k) in enumerate(slabs_b):
            hi = k if h == 0 else k + 1
            dst = (slabs_b[h + 1][0][:, 0, :] if h + 1 < len(slabs_b)
                   else qslab[:, 0, :])
            src = slab[:, 0:hi, :].rearrange("n t d -> n d t")
            nc.vector.reduce_sum(dst, src, axis=AX.X)

        ssum = work.tile([M, D], FP32, tag="ssum")
        GT = psum_g.tile([M, M], FP32, tag="GT")
        sth4 = small.tile([M, nch], FP32, tag="sth4")
        st_s = small.tile([M, 1], FP32, tag="st_s")

        # closing per-d-quarter reduces (partial slot + LAST_T slices each)
        for c in range(nch):
            dq = slice(c * 128, (c + 1) * 128)
            src = qslab[:, 0:1 + LAST_T, dq].rearrange("n t d -> n d t")
            nc.vector.reduce_sum(ssum[:, dq], src, axis=AX.X)
        # sT copies ride on DVE for the last batch (ACT is busy with squares
        # and DVE is idle after the closing reduces); ACT otherwise.
        sT_copy = nc.vector.tensor_copy if last else nc.scalar.copy
        # per-quarter transposes (PE picks these up as each reduce lands)
        pts = []
        for c in range(nch):
            pt = psum_t.tile([128, M], FP32, tag="pt", name=f"ps{c}")
            nc.tensor.transpose(pt[:], ssum[:, c * 128:(c + 1) * 128],
                                ident_sb[:, 0:M])
            pts.append(pt)
        # per-quarter squared-norm contributions (ACT), merged below
        for c in range(nch):
            dq = slice(c * 128, (c + 1) * 128)
            sq_scr = work.tile([M, D], FP32, tag="sq_scr")
            nc.scalar.activation(sq_scr[:, dq], ssum[:, dq], AF.Square,
                                 accum_out=sth4[:, c:c + 1])
        s4_scr = small.tile([M, nch], FP32, tag="s4_scr")
        nc.scalar.activation(s4_scr[:], sth4[:], AF.Copy, accum_out=st_s[:])
        # rs_s = 1/sqrt(st) = exp(-0.5*ln(st)), written straight into vrg
        ln_s = small.tile([M, 1], FP32, tag="ln_s")
        nc.scalar.activation(ln_s[:], st_s[:], AF.Ln)
        nc.scalar.activation(vrg[:, 2 * b:2 * b + 1], ln_s[:], AF.Exp,
                             scale=-0.5)
        # sT copies + GT accumulation: GT[n,m] = sum_d sT[d,n] * tT[d,m]
        for c in range(nch):
            sT = sbt.tile([128, M], FP32, tag="sT", name=f"sT{c}")
            sT_copy(_mm(sT[:]), pts[c][:])
            nc.tensor.matmul(GT[:], _mm(sT[:]), _mm(t_chunks[c][:]),
                             start=(c == 0), stop=(c == nch - 1))

        # ---- diag(GT) -> vrg; E = exp(rs_s * GT); se_row = ones^T @ E ------
        gd_scr = work.tile([M, M], FP32, tag="gd_scr")
        nc.vector.scalar_tensor_tensor(
            gd_scr[:], GT[:], 1.0, ident_sb[:, 0:M],
            op0=OP.mult, op1=OP.mult, accum_out=vrg[:, 2 * b + 1:2 * b + 2],
        )
        E = work.tile([M, M], FP32, tag="E")
        nc.scalar.activation(E[:], GT[:], AF.Exp,
                             scale=vrg[:, 2 * b:2 * b + 1])
        # plain fp32 (2-pass) matmul: E is an ACT output, not f32r-rounded,
        # and the [1,M] row costs ~nothing either way.
        se_ps = psum_se.tile([1, M], FP32, tag="se")
        nc.tensor.matmul(se_ps[:], ones_col[:], E[:], start=True, stop=True)
        se_sb = small.tile([1, M], FP32, tag="se_sb")
        nc.vector.tensor_copy(se_sb[:], se_ps[:])
        if last:
            # rg first (tiny, ready earlier), then the final se row
            nc.scalar.dma_start(rg[:, :], vrg[:])
            nc.sync.dma_start(seR[b:b + 1, 0:M], se_sb[:])
        else:
            nc.scalar.dma_start(seR[b:b + 1, 0:M], se_sb[:])


def _build_nc():
    nc = bacc.Bacc("TRN2", debug=False)
    skel = nc.dram_tensor("skel", [BPC, M, T, D], FP32, kind="ExternalInput")
    text = nc.dram_tensor("text", [BPC, M, D], FP32, kind="ExternalInput")
    ident = nc.dram_tensor("ident", [M, 128], FP32, kind="ExternalInput")
    seR = nc.dram_tensor("seR", [BPC, 128], FP32, kind="ExternalOutput")
    rg = nc.dram_tensor("rg", [M, 2 * BPC], FP32, kind="ExternalOutput")
    with tile.TileContext(nc) as tc, ExitStack() as ctx:
        _emit(tc, ctx, skel.ap(), text.ap(), ident.ap(), seR.ap(), rg.ap())
    with _patched_act_tables():
        nc.compile()
    return nc


_NC_CACHE = []


def _run(skeleton_embeddings, text_embeddings, **kw):
    if not _NC_CACHE:
        _NC_CACHE.append(_build_nc())
    nc = _NC_CACHE[0]
    skel = np.ascontiguousarray(np.asarray(skeleton_embeddings, dtype=np.float32))
    text = np.ascontiguousarray(np.asarray(text_embeddings, dtype=np.float32))
    ident = np.zeros((M, 128), dtype=np.float32)
    ident[np.arange(M), np.arange(M)] = 1.0
    in_maps = [
        {
            "skel": skel[c * BPC:(c + 1) * BPC],
            "text": text[c * BPC:(c + 1) * BPC],
            "ident": ident,
        }
        for c in range(NCORES)
    ]
    r = run_bass_kernel_spmd(nc, in_maps, core_ids=list(range(NCORES)), **kw)
    # loss_b = sum_m ln(se[b,m]) - sum_m rs[m,b]*gdiag[m,b]; mean over b, m
    total = 0.0
    for m_ in r.results:
        se = np.asarray(m_["seR"][:, 0:M], dtype=np.float64)
        v = np.asarray(m_["rg"], dtype=np.float64)
        rs = v[:, 0::2]   # [M, BPC]
        gd = v[:, 1::2]   # [M, BPC]
        total += float(np.log(se).sum() - (rs * gd).sum())
    loss = np.float32(total / (B * M))
    return loss, r


def kernel(skeleton_embeddings, text_embeddings):
    loss, _ = _run(skeleton_embeddings, text_embeddings)
    return np.asarray(loss, dtype=np.float32)
